# revision 38
# baseline (speedup 1.0000x reference)
"""2-layer GAT (gnn_message_passing) on 8 Trainium2 NeuronCores.

Strategy (per sharding hint): nodes are partitioned contiguously across the 8
cores (12500 each). Edges (incl. self-loops) are sharded by destination core,
sorted by destination window (128 dst nodes) and source range (quarter of the
node space, so gather indices fit int16), and padded to a static tile grid.

Per layer: a dense phase computes per-node transformed features
hp = x @ W and attention logit halves al_src/al_dst (folded into the weight
matrix as extra columns), writes them into a 256B-strided row table, and an
AllGather replicates the table to every core. The edge phase gathers rows by
edge source (custom dma_gather), gathers al_dst (bf16, 16B rows of the local
aldsel table) by edge destination, forms ee = exp(leaky_relu(al_s + al_d)) per
edge (lrelu on DVE so ACT keeps one table), multiplies messages, and
scatter-adds per destination window with a one-hot selection matmul on the PE
(which also accumulates the softmax denominators). Output rows are contiguous
per window, so no scatter is needed on the way out.

Perf notes (measured on HW via pipelined-dispatch slope timing):
- SWDGE dma_gather costs ~8 ns/descriptor PER QUEUE regardless of elem size
  (16B..272B); descriptor count, not bytes, is the cost. 4 SWDGE queues run
  in parallel (num_swdge_queues=4) -> ~2 ns/desc aggregate. Gathers are
  spread: hp on queue rr (per source range), al_dst on (rr+2)%4.
- single_packet=True wedges the device; never use it.
- Group epilogues are batched (ELU / log_softmax / table writes once per
  5-window group, not per window) to avoid ACT table swaps (Exp<->Ln, 1.3us
  each) and per-window small DMAs.
- One-hot sel matrices are built on DVE (is_equal vs iota, bf16). Gathering
  precomputed one-hot rows instead costs +72MB/layer of gather bandwidth and
  is a net loss (measured).
- History: naive 10.6 ms -> queues 5.9 -> batched epilogues + bf16 4.8 ms.
"""
import math
import numpy as np
import ml_dtypes

import concourse.bacc as bacc
import concourse.mybir as mybir
import concourse.tile as tile
from concourse import ap_utils

bf16 = ml_dtypes.bfloat16
F32 = mybir.dt.float32
BF16 = mybir.dt.bfloat16
I16 = mybir.dt.int16
I32 = mybir.dt.int32

P = 128
MAX_IDX_PER_GATHER = 3840   # DMA desc ring: <=~4080 idxs per gather inst
SLOPE = 0.2


# ---------------------------------------------------------------- dma_gather
def dma_gather_raw(eng, out_ap, in_ap, idxs_ap, num_idxs, elem_size,
                   elem_step=None, queue_num=0, single_packet=False):
    """BassGpSimd.dma_gather (DRAM src, non-transpose) minus the
    elem_size%256B assert (transpose-only restriction, see q7 source) and
    with single_packet=False (large single packets wedge the SDMA)."""
    assert idxs_ap.dtype == mybir.dt.int16
    assert in_ap.dtype == out_ap.dtype
    elem_size_bytes = elem_size * mybir.dt.size(in_ap.dtype)
    assert elem_size_bytes > 0
    if elem_step is None:
        elem_step = elem_size
    assert ap_utils.ap_is_contiguous(in_ap.ap[1:])
    assert ap_utils.ap_is_contiguous(out_ap.ap[1:])
    assert ap_utils.ap_is_contiguous(idxs_ap.ap[1:])
    assert in_ap.ap[0][0] == elem_step
    assert in_ap.ap[-1][1] == elem_size
    assert out_ap.ap[-1][1] == elem_size
    assert num_idxs <= MAX_IDX_PER_GATHER + 256
    stride_bytes = elem_step * mybir.dt.size(in_ap.dtype)
    assert stride_bytes % 256 == 0 and stride_bytes // 256 < 256
    _in_ap = eng.lower_ap_dma(in_ap, for_custom_bir_dma=True)
    _idxs_ap = eng.lower_ap(idxs_ap)
    _out_ap = eng.lower_ap(out_ap)
    return eng.add_instruction(
        mybir.InstDMAGatherAnt(
            name=eng.bass.get_next_instruction_name(),
            ins=[*_in_ap, _idxs_ap, eng.lower_val_access(eng.to_reg(num_idxs))],
            outs=[_out_ap],
            transpose=False,
            num_idxs=num_idxs,
            elem_size=elem_size,
            stride_bytes_256=stride_bytes // 256,
            gen_mode=0,
            single_packet=single_packet,
            queue_num=queue_num,
            sbuf_tokens_per_rank=0,
            sbuf_free_dim_per_rank=0,
            sbuf_free_dim_pad_per_rank=0,
            sbuf_byte_offset=0,
        )
    )


# ------------------------------------------------------------- host preproc
def _wrap_idx(seq16):
    """[NWIN, L] int -> [NWIN, 128, L//16] int16 in dma_gather idx layout
    (idx j at lane j%16 col j//16, replicated to 8 lane groups)."""
    nw, L = seq16.shape
    w = seq16.reshape(nw, L // 16, 16).transpose(0, 2, 1)      # [NWIN,16,K]
    w = np.tile(w, (1, 8, 1))                                  # [NWIN,128,K]
    return np.ascontiguousarray(w.astype(np.int16))


def preprocess(edge_index, cfg):
    """Sort/pad edges into the static (core, window, range, tile, lane) grid."""
    N, ncores, nloc, nwin, nrange = (cfg["N"], cfg["ncores"], cfg["nloc"],
                                     cfg["nwin"], cfg["nrange"])
    rng_sz = N // nrange
    loops = np.arange(N, dtype=np.int64)
    src = np.concatenate([edge_index[0].astype(np.int64), loops])
    dst = np.concatenate([edge_index[1].astype(np.int64), loops])
    core = dst // nloc
    dst_loc = dst - core * nloc
    w = dst_loc // P
    dst_rel = dst_loc - w * P
    r = src // rng_sz
    src_rel = src - r * rng_sz
    key = (core * nwin + w) * nrange + r
    counts = np.bincount(key, minlength=ncores * nwin * nrange)
    t_r = max(1, math.ceil(counts.max() / P))
    run = t_r * P
    order = np.argsort(key, kind="stable")
    ks = key[order]
    starts = np.zeros(ncores * nwin * nrange + 1, np.int64)
    np.cumsum(counts, out=starts[1:])
    pos = np.arange(len(ks)) - starts[ks]
    slot = ks * run + pos
    tot = ncores * nwin * nrange * run
    srcrel_f = np.zeros(tot, np.int16)
    dstrel_f = np.full(tot, -1.0, bf16)
    # pad slots point at the zero rows [nloc, nloc+128) of the aldsel table
    dstloc_f = np.full(tot, nloc, np.int16)
    srcrel_f[slot] = src_rel[order].astype(np.int16)
    dstrel_f[slot] = dst_rel[order].astype(np.float32)
    dstloc_f[slot] = dst_loc[order].astype(np.int16)
    srcrel_f = srcrel_f.reshape(ncores, nwin, nrange, run)
    dstrel_f = dstrel_f.reshape(ncores, nwin, nrange, t_r, P)
    dstloc_f = dstloc_f.reshape(ncores, nwin, nrange, run)

    per_core = []
    for c in range(ncores):
        m = {}
        for rr in range(nrange):
            # [128, NWIN, K] idx layouts (K = t_r*8)
            m[f"iA{rr}"] = np.ascontiguousarray(
                _wrap_idx(srcrel_f[c, :, rr, :]).transpose(1, 0, 2))
            m[f"iB{rr}"] = np.ascontiguousarray(
                _wrap_idx(dstloc_f[c, :, rr, :]).transpose(1, 0, 2))
        # dstrel device layout [128, NWIN, nrange*t_r]
        m["dstrel"] = np.ascontiguousarray(
            dstrel_f[c].transpose(3, 0, 1, 2).reshape(P, nwin, nrange * t_r))
        per_core.append(m)
    return per_core, t_r


# ------------------------------------------------------------- device build
ALL_PARTS = ("d1", "ag1", "e1g", "e1c", "e1s", "ag2", "e2g", "e2c", "e2s")


def build_nc(cfg, t_r, parts=ALL_PARTS, inner_reps=1):
    parts = frozenset(parts)
    N, ncores, nloc, nwin, nrange = (cfg["N"], cfg["ncores"], cfg["nloc"],
                                     cfg["nwin"], cfg["nrange"])
    F_IN, H1, C1, C2 = cfg["F_IN"], cfg["H1"], cfg["C1"], cfg["C2"]
    D1 = H1 * C1                   # 64
    A1 = D1 + 2 * H1               # 80: [hp | al_s | al_d]
    T1W = D1 + H1                  # 72 table row (hp | al_s)
    D2 = C2                        # 16
    T2W = D2 + 2                   # 18 table row (hp2 | al_s2 | pad)
    K = t_r * 8                    # idx cols per window
    G = max(1, MAX_IDX_PER_GATHER // (t_r * P))     # windows per group
    n_groups = math.ceil(nwin / G)
    kchunks = F_IN // P
    last_rows = nloc - (nwin - 1) * P

    nc = bacc.Bacc("TRN2", target_bir_lowering=False, num_devices=ncores,
                   num_swdge_queues=4)
    xT = nc.dram_tensor("xT", [F_IN, nloc], BF16, kind="ExternalInput")
    W1e = nc.dram_tensor("W1e", [F_IN, A1], BF16, kind="ExternalInput")
    W2e = nc.dram_tensor("W2e", [D1, T2W], BF16, kind="ExternalInput")
    b1r = nc.dram_tensor("b1r", [P, D1], F32, kind="ExternalInput")
    b2r = nc.dram_tensor("b2r", [P, D2], F32, kind="ExternalInput")
    iA = [nc.dram_tensor(f"iA{rr}", [P, nwin, K], I16, kind="ExternalInput")
          for rr in range(nrange)]
    iB = [nc.dram_tensor(f"iB{rr}", [P, nwin, K], I16, kind="ExternalInput")
          for rr in range(nrange)]
    # tiled identity rows (+ 128 zero rows for pad slots) for the aldsel tables
    osel = nc.dram_tensor("osel", [nloc + P, P], BF16, kind="ExternalInput")
    dstrel = nc.dram_tensor("dstrel", [P, nwin, nrange * t_r], BF16,
                            kind="ExternalInput")
    out = nc.dram_tensor("out", [nloc, D2], F32, kind="ExternalOutput")
    SEL_W = nrange * t_r * P       # sel columns per window

    ASW = 256                      # aldsel row stride (elements)
    ASE = 8 + P                    # aldsel gathered row: [al_d x8 | onehot x128]

    with tile.TileContext(nc) as tc:
        with (
            tc.tile_pool(name="const", bufs=1) as cpool,
            tc.tile_pool(name="sbuf", bufs=3) as sb,
            tc.tile_pool(name="gat", bufs=4) as gp,
            tc.tile_pool(name="psum", bufs=2, space="PSUM") as ps,
            tc.tile_pool(name="psum1", bufs=2, space="PSUM") as ps1,
            tc.tile_pool(name="dram", bufs=1, space="DRAM") as dr,
        ):
            t1loc = dr.tile([nloc, P], BF16)
            t1full = dr.tile([N, P], BF16)
            aldsel1 = dr.tile([nloc + P, ASW], BF16)
            t2loc = dr.tile([nloc, P], BF16)
            t2full = dr.tile([N, P], BF16)
            aldsel2 = dr.tile([nloc + P, ASW], BF16)

            # ---- static constants
            w1s = cpool.tile([P, kchunks, A1], BF16)
            nc.sync.dma_start(out=w1s[:], in_=W1e[:].rearrange(
                "(c p) a -> p c a", p=P))
            w2s = cpool.tile([D1, T2W], BF16)
            nc.sync.dma_start(out=w2s[:], in_=W2e[:])
            b1s = cpool.tile([P, D1], F32)
            nc.sync.dma_start(out=b1s[:], in_=b1r[:])
            b2s = cpool.tile([P, D2], F32)
            nc.sync.dma_start(out=b2s[:], in_=b2r[:])
            ident = cpool.tile([P, P], F32)
            from concourse.masks import make_identity
            make_identity(nc, ident[:])
            iota_i = cpool.tile([P, SEL_W], I32)
            nc.gpsimd.iota(iota_i[:], pattern=[[0, nrange * t_r], [1, P]],
                           base=0, channel_multiplier=0)
            iota_f = cpool.tile([P, SEL_W], BF16)
            nc.vector.tensor_copy(out=iota_f[:], in_=iota_i[:])
            # zero the al_d columns of the pad rows
            z8 = cpool.tile([P, 8], BF16)
            nc.vector.memset(z8[:], 0.0)
            nc.sync.dma_start(out=aldsel1[nloc:nloc + P, 0:8], in_=z8[:])
            nc.sync.dma_start(out=aldsel2[nloc:nloc + P, 0:8], in_=z8[:])

            # ---- phase A: tables for layer 1
            def dense_phase():
                for b in range(nwin):
                    r0 = b * P
                    rows = P if b < nwin - 1 else last_rows
                    xa = sb.tile([P, kchunks, P], BF16, tag="xa")
                    nc.sync.dma_start(
                        out=xa[:, :, :rows],
                        in_=xT[:, r0:r0 + rows].rearrange("(c p) r -> p c r",
                                                          p=P))
                    pA = ps.tile([P, A1], F32, tag="pA")
                    for c in range(kchunks):
                        nc.tensor.matmul(pA[:], lhsT=xa[:, c, :],
                                         rhs=w1s[:, c, :],
                                         start=(c == 0),
                                         stop=(c == kchunks - 1))
                    t1row = sb.tile([P, T1W], BF16, tag="t1row")
                    nc.vector.tensor_copy(out=t1row[:], in_=pA[:, 0:T1W])
                    a1row = sb.tile([P, H1], BF16, tag="a1row")
                    nc.scalar.copy(out=a1row[:], in_=pA[:, T1W:A1])
                    nc.sync.dma_start(out=t1loc[r0:r0 + rows, 0:T1W],
                                      in_=t1row[:rows, :])
                    nc.sync.dma_start(out=aldsel1[r0:r0 + rows, 0:H1],
                                      in_=a1row[:rows, :])

            # ---- edge phases
            def edge_phase(layer, do_g=True, do_c=True, do_s=True):
                tfull = t1full if layer == 1 else t2full
                ald = aldsel1 if layer == 1 else aldsel2
                TW = T1W if layer == 1 else T2W
                DH = D1 if layer == 1 else D2       # message width
                NH = H1 if layer == 1 else 1        # heads
                CH = DH // NH
                rng_rows = N // nrange
                for g in range(n_groups):
                    w0 = g * G
                    Gg = min(G, nwin - w0)
                    nidx = Gg * t_r * P
                    hp_g, ald_g = [], []
                    for rr in range(nrange):
                        hg = gp.tile([P, G * t_r, TW], BF16, tag=f"hg{rr}")
                        hp_g.append(hg)
                        ag = gp.tile([P, G * t_r, 8], BF16, tag=f"ag{rr}")
                        ald_g.append(ag)
                        if not do_g:
                            # timing-only variant: mark tiles written
                            nc.vector.memset(hg[:, 0:1, 0:2], 1.0)
                            nc.vector.memset(ag[:, 0:1, 0:2], 1.0)
                            continue
                        it = sb.tile([P, G, K], I16, tag=f"iA{rr}")
                        nc.sync.dma_start(out=it[:, :Gg, :],
                                          in_=iA[rr][:, w0:w0 + Gg, :])
                        dma_gather_raw(
                            nc.gpsimd, hg[:, :Gg * t_r, :],
                            tfull[rr * rng_rows:(rr + 1) * rng_rows, 0:TW],
                            it[:, :Gg, :].rearrange("p g k -> p (g k)"),
                            nidx, TW, elem_step=P, queue_num=rr)
                        it2 = sb.tile([P, G, K], I16, tag=f"iB{rr}")
                        nc.sync.dma_start(out=it2[:, :Gg, :],
                                          in_=iB[rr][:, w0:w0 + Gg, :])
                        # fetches al_d (8 cols) per edge slot
                        dma_gather_raw(
                            nc.gpsimd, ag[:, :Gg * t_r, :],
                            ald[:, 0:8],
                            it2[:, :Gg, :].rearrange("p g k -> p (g k)"),
                            nidx, 8, elem_step=ASW, queue_num=(rr + 2) % 4)
                    dre = sb.tile([P, G, nrange * t_r], BF16, tag="dre")
                    if do_s:
                        nc.sync.dma_start(out=dre[:, :Gg, :],
                                          in_=dstrel[:, w0:w0 + Gg, :])

                    # ee = exp(lrelu(al_s + al_d)) for the whole group
                    if do_c:
                        zz = gp.tile([P, nrange, G * t_r, NH], F32, tag="zz")
                        for rr in range(nrange):
                            nc.vector.tensor_tensor(
                                out=zz[:, rr, :Gg * t_r, :],
                                in0=hp_g[rr][:, :Gg * t_r, DH:DH + NH],
                                in1=ald_g[rr][:, :Gg * t_r, 0:NH],
                                op=mybir.AluOpType.add)
                        zzf = zz[:].rearrange("p r t h -> p (r t h)")
                        # lrelu on DVE (avoids ACT table swap Lrelu<->Exp)
                        zt = gp.tile([P, nrange, G * t_r, NH], F32, tag="zt")
                        ztf = zt[:].rearrange("p r t h -> p (r t h)")
                        nc.vector.tensor_scalar(
                            out=ztf, in0=zzf, scalar1=SLOPE, scalar2=None,
                            op0=mybir.AluOpType.mult)
                        nc.vector.tensor_tensor(
                            out=zzf, in0=zzf, in1=ztf,
                            op=mybir.AluOpType.max)
                        nc.scalar.activation(
                            out=zzf, in_=zzf,
                            func=mybir.ActivationFunctionType.Exp)
                        # messages in place: hp *= ee ; al_s cols := ee
                        for rr in range(nrange):
                            nc.vector.tensor_tensor(
                                out=hp_g[rr][:, :Gg * t_r, 0:DH].rearrange(
                                    "p t (h c) -> p t h c", h=NH),
                                in0=hp_g[rr][:, :Gg * t_r, 0:DH].rearrange(
                                    "p t (h c) -> p t h c", h=NH),
                                in1=zz[:, rr, :Gg * t_r, :, None].to_broadcast(
                                    [P, Gg * t_r, NH, CH]),
                                op=mybir.AluOpType.mult)
                            nc.vector.tensor_copy(
                                out=hp_g[rr][:, :Gg * t_r, DH:DH + NH],
                                in_=zz[:, rr, :Gg * t_r, :])

                    if not do_s:
                        continue
                    hacc = sb.tile([P, G, DH], F32, tag="hacc")
                    for wl in range(Gg):
                        sel = sb.tile([P, nrange * t_r, P], BF16, tag="sel")
                        nc.vector.tensor_tensor(
                            out=sel[:],
                            in0=iota_f[:].rearrange("p (t q) -> p t q", q=P),
                            in1=dre[:, wl, :, None].to_broadcast(
                                [P, nrange * t_r, P]),
                            op=mybir.AluOpType.is_equal)
                        acc = ps.tile([P, DH + NH], F32, tag="acc")
                        nmm = nrange * t_r
                        i = 0
                        for rr in range(nrange):
                            for t in range(t_r):
                                nc.tensor.matmul(
                                    acc[:],
                                    lhsT=sel[:, rr * t_r + t, :],
                                    rhs=hp_g[rr][:, wl * t_r + t, 0:DH + NH],
                                    start=(i == 0), stop=(i == nmm - 1))
                                i += 1
                        rec = sb.tile([P, NH], F32, tag="rec")
                        nc.vector.reciprocal(out=rec[:], in_=acc[:, DH:DH + NH])
                        nc.vector.tensor_tensor(
                            out=hacc[:, wl, :].rearrange("p (h c) -> p h c",
                                                         h=NH),
                            in0=acc[:, 0:DH].rearrange("p (h c) -> p h c",
                                                       h=NH),
                            in1=rec[:, :, None].to_broadcast([P, NH, CH]),
                            op=mybir.AluOpType.mult)
                        nc.vector.tensor_tensor(
                            out=hacc[:, wl, :], in0=hacc[:, wl, :],
                            in1=(b1s if layer == 1 else b2s)[:],
                            op=mybir.AluOpType.add)

                    # ---- group-level epilogue (batched ACT + DMA)
                    n_full = Gg if w0 + Gg < nwin else Gg - 1
                    if layer == 1:
                        hf = hacc[:, :Gg, :].rearrange("p g d -> p (g d)")
                        t1 = sb.tile([P, G, DH], F32, tag="elu1")
                        t1f = t1[:, :Gg, :].rearrange("p g d -> p (g d)")
                        nc.vector.tensor_scalar(
                            out=t1f, in0=hf, scalar1=0.0, scalar2=-1.0,
                            op0=mybir.AluOpType.max, op1=mybir.AluOpType.add)
                        t2 = sb.tile([P, G, DH], F32, tag="elu2")
                        t2f = t2[:, :Gg, :].rearrange("p g d -> p (g d)")
                        nc.vector.tensor_scalar_min(out=t2f, in0=hf,
                                                    scalar1=0.0)
                        nc.scalar.activation(
                            out=t2f, in_=t2f,
                            func=mybir.ActivationFunctionType.Exp)
                        nc.vector.tensor_tensor(out=hf, in0=t1f, in1=t2f,
                                                op=mybir.AluOpType.add)
                        t2acc = sb.tile([P, G, T2W], BF16, tag="t2acc")
                        a2acc = sb.tile([P, G, 1], BF16, tag="a2acc")
                        for wl in range(Gg):
                            hTp = ps1.tile([D1, P], F32, tag="hTp")
                            nc.tensor.transpose(out=hTp[:], in_=hacc[:, wl, :],
                                                identity=ident[:])
                            hTb = sb.tile([D1, P], BF16, tag="hTb")
                            nc.vector.tensor_copy(out=hTb[:], in_=hTp[:])
                            p2 = ps1.tile([P, T2W], F32, tag="p2")
                            nc.tensor.matmul(p2[:], lhsT=hTb[:], rhs=w2s[:],
                                             start=True, stop=True)
                            nc.vector.tensor_copy(out=t2acc[:, wl, :],
                                                  in_=p2[:])
                            nc.scalar.copy(out=a2acc[:, wl, :],
                                           in_=p2[:, D2 + 1:D2 + 2])
                        if n_full > 0:
                            nc.sync.dma_start(
                                out=t2loc[w0 * P:(w0 + n_full) * P, 0:T2W]
                                .rearrange("(g p) d -> p g d", p=P),
                                in_=t2acc[:, :n_full, :])
                            nc.sync.dma_start(
                                out=aldsel2[w0 * P:(w0 + n_full) * P, 0:1]
                                .rearrange("(g p) d -> p g d", p=P),
                                in_=a2acc[:, :n_full, :])
                        if w0 + Gg == nwin:
                            wl = Gg - 1
                            r0 = (w0 + wl) * P
                            nc.sync.dma_start(
                                out=t2loc[r0:r0 + last_rows, 0:T2W],
                                in_=t2acc[:last_rows, wl, :])
                            nc.sync.dma_start(
                                out=aldsel2[r0:r0 + last_rows, 0:1],
                                in_=a2acc[:last_rows, wl, :])
                    else:
                        mxg = sb.tile([P, G], F32, tag="mxg")
                        nc.vector.tensor_reduce(
                            out=mxg[:, :Gg], in_=hacc[:, :Gg, :],
                            axis=mybir.AxisListType.X, op=mybir.AluOpType.max)
                        ttg = sb.tile([P, G, D2], F32, tag="ttg")
                        nc.vector.tensor_tensor(
                            out=ttg[:, :Gg, :], in0=hacc[:, :Gg, :],
                            in1=mxg[:, :Gg, None].to_broadcast([P, Gg, D2]),
                            op=mybir.AluOpType.subtract)
                        exg = sb.tile([P, G, D2], F32, tag="exg")
                        nc.scalar.activation(
                            out=exg[:, :Gg, :].rearrange("p g d -> p (g d)"),
                            in_=ttg[:, :Gg, :].rearrange("p g d -> p (g d)"),
                            func=mybir.ActivationFunctionType.Exp)
                        sg = sb.tile([P, G], F32, tag="sg")
                        nc.vector.tensor_reduce(
                            out=sg[:, :Gg], in_=exg[:, :Gg, :],
                            axis=mybir.AxisListType.X, op=mybir.AluOpType.add)
                        lsg = sb.tile([P, G], F32, tag="lsg")
                        nc.scalar.activation(
                            out=lsg[:, :Gg], in_=sg[:, :Gg],
                            func=mybir.ActivationFunctionType.Ln)
                        nc.vector.tensor_tensor(
                            out=ttg[:, :Gg, :], in0=ttg[:, :Gg, :],
                            in1=lsg[:, :Gg, None].to_broadcast([P, Gg, D2]),
                            op=mybir.AluOpType.subtract)
                        if n_full > 0:
                            nc.sync.dma_start(
                                out=out[w0 * P:(w0 + n_full) * P, :]
                                .rearrange("(g p) d -> p g d", p=P),
                                in_=ttg[:, :n_full, :])
                        if w0 + Gg == nwin:
                            wl = Gg - 1
                            r0 = (w0 + wl) * P
                            nc.sync.dma_start(
                                out=out[r0:r0 + last_rows, :],
                                in_=ttg[:last_rows, wl, :])

            for _rep in range(inner_reps):
                if "d1" in parts:
                    dense_phase()
                if "ag1" in parts:
                    nc.gpsimd.collective_compute(
                        "AllGather", mybir.AluOpType.bypass,
                        replica_groups=[list(range(ncores))],
                        ins=[t1loc[:].opt()], outs=[t1full[:].opt()])
                if parts & {"e1g", "e1c", "e1s"}:
                    edge_phase(1, do_g="e1g" in parts, do_c="e1c" in parts,
                               do_s="e1s" in parts)
                if "ag2" in parts:
                    nc.gpsimd.collective_compute(
                        "AllGather", mybir.AluOpType.bypass,
                        replica_groups=[list(range(ncores))],
                        ins=[t2loc[:].opt()], outs=[t2full[:].opt()])
                if parts & {"e2g", "e2c", "e2s"}:
                    edge_phase(2, do_g="e2g" in parts, do_c="e2c" in parts,
                               do_s="e2s" in parts)

    nc.compile()
    return nc


# ------------------------------------------------------------------ runner
class SpmdRunner:
    def __init__(self, nc, n_cores):
        import jax
        from jax.sharding import Mesh, PartitionSpec
        from jax.experimental.shard_map import shard_map
        from concourse.bass2jax import (_bass_exec_p, partition_id_tensor,
                                        install_neuronx_cc_hook)
        install_neuronx_cc_hook()
        self.jax = jax
        self.n_cores = n_cores
        pname = nc.partition_id_tensor.name if nc.partition_id_tensor else None
        in_names, out_names, out_avals, zero_outs = [], [], [], []
        for alloc in nc.m.functions[0].allocations:
            if not isinstance(alloc, mybir.MemoryLocationSet):
                continue
            name = alloc.memorylocations[0].name
            if alloc.kind == "ExternalInput":
                if name != pname:
                    in_names.append(name)
            elif alloc.kind == "ExternalOutput":
                out_names.append(name)
                out_avals.append(jax.core.ShapedArray(
                    tuple(alloc.tensor_shape), mybir.dt.np(alloc.dtype)))
                zero_outs.append(np.zeros(tuple(alloc.tensor_shape),
                                          mybir.dt.np(alloc.dtype)))
        self.in_names, self.out_names = in_names, out_names
        self.out_avals, self.zero_outs = out_avals, zero_outs
        self.n_params = len(in_names)
        all_in = in_names + out_names + ([pname] if pname else [])

        def _body(*args):
            operands = list(args)
            if pname is not None:
                operands.append(partition_id_tensor())
            return tuple(_bass_exec_p.bind(
                *operands, out_avals=tuple(out_avals), in_names=tuple(all_in),
                out_names=tuple(out_names), lowering_input_output_aliases=(),
                sim_require_finite=True, sim_require_nnan=True, nc=nc))

        donate = tuple(range(self.n_params, self.n_params + len(out_avals)))
        devices = jax.devices()[:n_cores]
        self.mesh = Mesh(np.asarray(devices), ("core",))
        self.pspec = PartitionSpec("core")
        in_specs = (self.pspec,) * (self.n_params + len(out_avals))
        out_specs = (self.pspec,) * len(out_avals)
        self.sharded = jax.jit(
            shard_map(_body, mesh=self.mesh, in_specs=in_specs,
                      out_specs=out_specs, check_rep=False),
            donate_argnums=donate, keep_unused=True)

    def _stage_inputs(self, in_maps):
        from jax.sharding import NamedSharding
        jax = self.jax
        sh = NamedSharding(self.mesh, self.pspec)
        per_core = [[np.asarray(m[name]) for name in self.in_names]
                    for m in in_maps]
        concat = [np.concatenate([per_core[c][i] for c in range(self.n_cores)],
                                 axis=0) for i in range(self.n_params)]
        dev_in = [jax.device_put(a, sh) for a in concat]
        return sh, dev_in

    def _burst(self, dev_in, sh, k):
        """Dispatch k executions back-to-back (async), block at the end.
        Returns (wall_ns, out_arrs of last execution)."""
        import time
        jax = self.jax
        dzs = [[jax.device_put(
            np.zeros((self.n_cores * z.shape[0], *z.shape[1:]), z.dtype), sh)
            for z in self.zero_outs] for _ in range(k)]
        for dz in dzs:
            for a in dz:
                a.block_until_ready()
        t0 = time.perf_counter_ns()
        out_arrs = None
        for dz in dzs:
            out_arrs = self.sharded(*dev_in, *dz)
        for a in out_arrs:
            a.block_until_ready()
        return time.perf_counter_ns() - t0, out_arrs

    def run(self, in_maps, reps=1):
        """Correctness + best-wall timing (legacy contract)."""
        sh, dev_in = self._stage_inputs(in_maps)
        best = float("inf")
        out_arrs = None
        for _ in range(max(1, reps)):
            w, out_arrs = self._burst(dev_in, sh, 1)
            best = min(best, w)
        results = [
            {name: np.asarray(out_arrs[i]).reshape(
                self.n_cores, *self.out_avals[i].shape)[c]
             for i, name in enumerate(self.out_names)}
            for c in range(self.n_cores)]
        return results, best

    def run_slope(self, in_maps, k_small=3, k_big=13, rounds=3):
        """Pipelined-dispatch slope timing: per-execution ns estimated from
        (wall(k_big) - wall(k_small)) / (k_big - k_small), min over rounds."""
        sh, dev_in = self._stage_inputs(in_maps)
        self._burst(dev_in, sh, 1)          # warmup
        ws_min, wb_min = float("inf"), float("inf")
        out_arrs = None
        for _ in range(rounds):
            ws, _ = self._burst(dev_in, sh, k_small)
            wb, out_arrs = self._burst(dev_in, sh, k_big)
            ws_min, wb_min = min(ws_min, ws), min(wb_min, wb)
        per = (wb_min - ws_min) / (k_big - k_small)
        results = [
            {name: np.asarray(out_arrs[i]).reshape(
                self.n_cores, *self.out_avals[i].shape)[c]
             for i, name in enumerate(self.out_names)}
            for c in range(self.n_cores)]
        return results, per


# ----------------------------------------------------------------- kernel()
def make_cfg(N, E, F_IN, H1, C1, C2, ncores):
    nloc = N // ncores
    return dict(N=N, E=E, F_IN=F_IN, H1=H1, C1=C1, C2=C2, ncores=ncores,
                nloc=nloc, nwin=math.ceil(nloc / P), nrange=4)


DEFAULT_CFG = make_cfg(N=100000, E=1600000, F_IN=512, H1=8, C1=8, C2=16,
                       ncores=8)


def fold_weights(W1, a1_src, a1_dst, W2, a2_src, a2_dst, cfg):
    H1, C1 = cfg["H1"], cfg["C1"]
    W1r = W1.reshape(cfg["F_IN"], H1, C1)
    w1s = np.einsum("khc,hc->kh", W1r, a1_src)
    w1d = np.einsum("khc,hc->kh", W1r, a1_dst)
    W1e = np.concatenate([W1, w1s, w1d], axis=1).astype(bf16)
    w2s = W2 @ a2_src[0]
    w2d = W2 @ a2_dst[0]
    W2e = np.concatenate([W2, w2s[:, None], w2d[:, None]], axis=1).astype(bf16)
    return W1e, W2e


_CACHE = {}


def prepare(inputs, cfg=DEFAULT_CFG, reps=1):
    x = np.asarray(inputs["x"], np.float32)
    edge_index = np.asarray(inputs["edge_index"])
    W1 = np.asarray(inputs["W1"], np.float32)
    W2 = np.asarray(inputs["W2"], np.float32)
    b1 = np.asarray(inputs["b1"], np.float32)
    b2 = np.asarray(inputs["b2"], np.float32)
    a1s = np.asarray(inputs["a1_src"], np.float32)
    a1d = np.asarray(inputs["a1_dst"], np.float32)
    a2s = np.asarray(inputs["a2_src"], np.float32)
    a2d = np.asarray(inputs["a2_dst"], np.float32)

    per_core_idx, t_r = preprocess(edge_index, cfg)
    key = (cfg["N"], t_r)
    if key not in _CACHE:
        nc = build_nc(cfg, t_r)
        _CACHE[key] = (nc, SpmdRunner(nc, cfg["ncores"]))
    nc, runner = _CACHE[key]

    W1e, W2e = fold_weights(W1, a1s, a1d, W2, a2s, a2d, cfg)
    b1rep = np.tile(b1[None, :], (P, 1)).astype(np.float32)
    b2rep = np.tile(b2[None, :], (P, 1)).astype(np.float32)
    nloc = cfg["nloc"]
    osel = np.zeros((nloc + P, P), bf16)
    rr = np.arange(nloc)
    osel[rr, rr % P] = 1.0
    in_maps = []
    for c in range(cfg["ncores"]):
        m = dict(per_core_idx[c])
        m["xT"] = np.ascontiguousarray(
            x[c * nloc:(c + 1) * nloc, :].T).astype(bf16)
        m["W1e"], m["W2e"] = W1e, W2e
        m["b1r"], m["b2r"] = b1rep, b2rep
        m["osel"] = osel
        in_maps.append(m)
    return runner, in_maps


def kernel_timed(inputs, reps=1):
    cfg = DEFAULT_CFG
    runner, in_maps = prepare(inputs, cfg, reps)
    results, best_ns = runner.run(in_maps, reps=reps)
    out = np.concatenate([results[c]["out"] for c in range(cfg["ncores"])],
                         axis=0)
    return out, best_ns


def kernel(**inputs):
    out, _ = kernel_timed(inputs, reps=1)
    return out

